# revision 32
# baseline (speedup 1.0000x reference)
"""Bahdanau-attention pooling kernel for TRN2, data-parallel over 8 NeuronCores.

Reference computation (per batch b):
    h   = tanh(enc @ W1enc.T + hid @ W1hid.T + b1)    [S, K]   (K = D = 512)
    e   = h @ w2                                       [S]
    a   = softmax(e)                                   [S]
    ctx = a @ enc                                      [D]

Distribution: batch dim (32) sharded 4-per-core across 8 cores; the small MLP
weights are replicated. No collectives needed.

Device algorithm (per core, single pass over the encoder stream):
  - encoder arrives pre-transposed [B_loc, D, S] so score matmuls contract
    over d on the partition dim with no on-chip transpose.
  - online (max-free) softmax: |e| <= ||w2||_1 ~= 11.3, so exp() never
    overflows fp32; accumulate unnormalized sum_s exp(e_s) * enc_s and
    z = sum_s exp(e_s), divide at the end.
  - per s-tile: 16 score MMs (k-chunk x d-chunk) -> tanh(+r bias) on ACT ->
    4 w2-MMs -> e row -> exp on ACT (accum_out gives the tile's z) ->
    replicate p across partitions with a K=1 ones-MM -> 4 fused
    multiply+reduce DVE ops accumulate the context.
"""

import numpy as np

B, S, D = 32, 4096, 512
N_CORES = 8
B_LOC = B // N_CORES
T = 512          # s-tile size
KC = D // 128    # 4 k-chunks
DC = D // 128    # 4 d-chunks


def build_nc(b_loc=B_LOC, s_len=S, t=T, dtype_name="float32"):
    import concourse.bass as bass
    import concourse.mybir as mybir
    import concourse.tile as tile

    fp32 = mybir.dt.float32
    bf16 = mybir.dt.bfloat16
    AF = mybir.ActivationFunctionType
    Alu = mybir.AluOpType

    nc = bass.Bass()

    enc_ext = nc.declare_dram_parameter("enc", [b_loc, D, s_len], bf16, isOutput=False)
    hid_ext = nc.declare_dram_parameter("hid", [b_loc, D], bf16, isOutput=False)
    w1et_ext = nc.declare_dram_parameter("w1et", [D, D], bf16, isOutput=False)
    w1ht_ext = nc.declare_dram_parameter("w1ht", [D, D], bf16, isOutput=False)
    b1_ext = nc.declare_dram_parameter("b1", [D], fp32, isOutput=False)
    w2_ext = nc.declare_dram_parameter("w2", [D], bf16, isOutput=False)
    out_ext = nc.declare_dram_parameter("out", [b_loc, D], fp32, isOutput=True)

    n_tiles = s_len // t

    with tile.TileContext(nc) as tc:
        with (
            tc.tile_pool(name="singles", bufs=1) as singles,
            tc.tile_pool(name="enc_pool", bufs=4) as enc_pool,
            tc.tile_pool(name="h_pool", bufs=8) as h_pool,
            tc.tile_pool(name="p_pool", bufs=3) as p_pool,
            tc.tile_pool(name="scr_pool", bufs=2) as scr_pool,
            tc.tile_pool(name="tiny", bufs=4) as tiny,
            tc.tile_pool(name="ps_h", bufs=4, space=bass.MemorySpace.PSUM) as ps_h,
            tc.tile_pool(name="ps_e", bufs=2, space=bass.MemorySpace.PSUM) as ps_e,
            tc.tile_pool(name="ps_s", bufs=1, space=bass.MemorySpace.PSUM) as ps_s,
            tc.tile_pool(name="pb_dram", bufs=4, space="DRAM") as pb_dram,
            tc.tile_pool(name="prep_pool", bufs=3) as prep_pool,
        ):
            # ---- persistent tiles (small loads first so r can start early) --
            hid_cols = singles.tile([128, b_loc, DC], bf16)  # [p(d), b, d-chunk]
            nc.gpsimd.dma_start(
                out=hid_cols, in_=hid_ext.rearrange("b (c p) -> p b c", p=128)
            )
            b1_col = singles.tile([128, KC], fp32)
            nc.gpsimd.dma_start(out=b1_col, in_=b1_ext.rearrange("(c p) -> p c", p=128))
            w2_col = singles.tile([128, KC], bf16)
            nc.gpsimd.dma_start(out=w2_col, in_=w2_ext.rearrange("(c p) -> p c", p=128))
            w1ht_sb = singles.tile([128, DC, D], bf16)
            nc.sync.dma_start(
                out=w1ht_sb, in_=w1ht_ext.rearrange("(c p) k -> p c k", p=128)
            )
            w1et_sb = singles.tile([128, DC, D], bf16)   # [p(d), d-chunk, k]
            nc.sync.dma_start(
                out=w1et_sb, in_=w1et_ext.rearrange("(c p) k -> p c k", p=128)
            )
            ones_row = singles.tile([1, 128], bf16)
            nc.vector.memset(ones_row, 1.0)
            ones_f32 = singles.tile([1, 128], fp32)
            nc.vector.memset(ones_f32, 1.0)

            r_sb = singles.tile([128, KC, b_loc], fp32)   # [p(k), k-chunk, b]
            c_acc = singles.tile([128, DC, b_loc], fp32)  # [p(d), d-chunk, b]
            z_acc = singles.tile([1, b_loc], fp32)
            nc.vector.memset(c_acc, 0.0)
            nc.vector.memset(z_acc, 0.0)

            # ---- r = W1hid @ hid + b1  (per k-chunk, all batches at once) ---
            for kc in range(KC):
                r_ps = ps_s.tile([128, b_loc], fp32, tag="s")
                for dc in range(DC):
                    nc.tensor.matmul(
                        r_ps,
                        w1ht_sb[:, dc, kc * 128:(kc + 1) * 128],
                        hid_cols[:, :, dc],
                        start=(dc == 0),
                        stop=(dc == DC - 1),
                    )
                nc.vector.tensor_scalar_add(
                    out=r_sb[:, kc, :], in0=r_ps, scalar1=b1_col[:, kc:kc + 1]
                )

            # ---- main loop: tiles processed in pairs so each LDWEIGHTS of a
            # (d-chunk, k-chunk) weight block feeds two matmuls -------------
            for b in range(b_loc):
                for it0 in range(0, n_tiles, 2):
                    encs = []
                    for j in range(2):
                        s0 = (it0 + j) * t
                        enc_t = enc_pool.tile([128, DC, t], bf16, tag="enc")
                        nc.sync.dma_start(
                            out=enc_t,
                            in_=enc_ext[b].rearrange("(c p) s -> p c s", p=128)[
                                :, :, s0:s0 + t
                            ],
                        )
                        encs.append(enc_t)

                    h_tiles = [[], []]
                    for kc in range(KC):
                        h_pss = [
                            ps_h.tile([128, t], fp32, tag="h", name=f"hps{j}")
                            for j in range(2)
                        ]
                        for dc in range(DC):
                            for j in range(2):
                                nc.tensor.matmul(
                                    h_pss[j],
                                    w1et_sb[:, dc, kc * 128:(kc + 1) * 128],
                                    encs[j][:, dc, :],
                                    start=(dc == 0),
                                    stop=(dc == DC - 1),
                                )
                        for j in range(2):
                            h_sb = h_pool.tile([128, t], bf16, tag="hsb")
                            nc.scalar.activation(
                                out=h_sb, in_=h_pss[j], func=AF.Tanh,
                                bias=r_sb[:, kc, b:b + 1], scale=1.0,
                            )
                            h_tiles[j].append(h_sb)

                    for j in range(2):
                        e_ps = ps_e.tile([1, t], fp32, tag="e")
                        for kc in range(KC):
                            nc.tensor.matmul(
                                e_ps,
                                w2_col[:, kc:kc + 1],
                                h_tiles[j][kc],
                                start=(kc == 0),
                                stop=(kc == KC - 1),
                            )

                        p_row = p_pool.tile([1, t], bf16, tag="p")
                        z_tile = tiny.tile([1, 1], fp32, tag="z")
                        nc.scalar.activation(
                            out=p_row, in_=e_ps, func=AF.Exp, accum_out=z_tile
                        )
                        nc.vector.tensor_tensor(
                            out=z_acc[:, b:b + 1], in0=z_acc[:, b:b + 1], in1=z_tile,
                            op=Alu.add,
                        )

                        # Replicate p across partitions without touching PE:
                        # bounce the 1KB row through DRAM, then re-load it with
                        # a partition-step-0 broadcast AP (legal from DRAM).
                        pb = pb_dram.tile([1, t], bf16, tag="pb")
                        nc.sync.dma_start(out=pb, in_=p_row)
                        p_rep = prep_pool.tile([128, t], bf16, tag="prep")
                        pb_bcast = bass.AP(
                            tensor=pb.tensor,
                            offset=pb.offset,
                            ap=[[0, 128]] + list(pb.ap)[1:],
                        )
                        nc.gpsimd.dma_start(out=p_rep, in_=pb_bcast)

                        for dc in range(DC):
                            scr = scr_pool.tile([128, t], bf16, tag="scr")
                            ctmp = tiny.tile([128, 1], fp32, tag="ct")
                            nc.vector.scalar_tensor_tensor(
                                out=scr,
                                in0=encs[j][:, dc, :],
                                scalar=1.0,
                                in1=p_rep,
                                op0=Alu.mult,
                                op1=Alu.mult,
                                accum_out=ctmp,
                            )
                            nc.vector.tensor_tensor(
                                out=c_acc[:, dc, b:b + 1],
                                in0=c_acc[:, dc, b:b + 1],
                                in1=ctmp,
                                op=Alu.add,
                            )

                # ---- batch epilogue -----------------------------------------
                zr = tiny.tile([1, 1], fp32)
                nc.vector.reciprocal(out=zr, in_=z_acc[:, b:b + 1])
                zr_ps = ps_s.tile([128, 1], fp32, tag="s")
                nc.tensor.matmul(zr_ps, ones_f32, zr, start=True, stop=True)
                out_t = tiny.tile([128, DC], fp32)
                nc.vector.tensor_scalar_mul(out=out_t, in0=c_acc[:, :, b], scalar1=zr_ps)
                nc.gpsimd.dma_start(
                    out=out_ext[b].rearrange("(c p) -> p c", p=128), in_=out_t
                )

    return nc


# Instruction opcodes whose ISA structs tolerate multi-waits (or that the
# split must not touch). Everything else on this walrus build has a single
# sync-wait slot, so excess waits move onto preceding same-engine NoOps.
_NO_SPLIT = {"EventSemaphore", "Call", "UnconditionalBranch", "RegisterMove"}


def split_multi_waits(nc, limit=1):
    import concourse.mybir as mybir

    ctr = 0
    for fn in nc.m.functions:
        for blk in fn.blocks:
            new = []
            for inst in blk.instructions:
                si = inst.sync_info
                waits = list(si.on_wait) if si is not None and si.on_wait else []
                if inst.opcode not in _NO_SPLIT and len(waits) > limit:
                    extra, keep = waits[:-limit], waits[-limit:]
                    for w in extra:
                        ctr += 1
                        new.append(mybir.InstNoOp(
                            name=f"WSPLIT-{ctr}", engine=inst.engine,
                            sync_info=mybir.SyncInfo(on_wait=[w], on_update=[])))
                    inst.sync_info = mybir.SyncInfo(
                        on_wait=keep,
                        on_update=list(si.on_update) if si.on_update else [])
                new.append(inst)
            blk.instructions = new
    return ctr


def _prep_host(hidden_state, encoder_output, W1, b1, w2):
    import ml_dtypes

    bf16 = ml_dtypes.bfloat16
    encT = np.ascontiguousarray(
        encoder_output.transpose(0, 2, 1).astype(bf16)
    )  # [B, D, S]
    w1et = np.ascontiguousarray(W1[:, :D].T.astype(bf16))   # [d, k]
    w1ht = np.ascontiguousarray(W1[:, D:].T.astype(bf16))   # [d, k]
    in_maps = []
    for i in range(N_CORES):
        sl = slice(i * B_LOC, (i + 1) * B_LOC)
        in_maps.append({
            "enc": np.ascontiguousarray(encT[sl]),
            "hid": np.ascontiguousarray(hidden_state[sl].astype(bf16)),
            "w1et": w1et,
            "w1ht": w1ht,
            "b1": np.ascontiguousarray(b1.astype(np.float32)),
            "w2": np.ascontiguousarray(w2.astype(bf16)),
        })
    return in_maps


def _ensure_ntff_hook():
    """Install the axon NTFF profile hook if the image lacks antenv.axon_hooks."""
    import sys
    import types

    try:
        import antenv.axon_hooks  # noqa: F401
        return
    except ImportError:
        pass
    import antenv

    mod = types.ModuleType("antenv.axon_hooks")
    state = {"hook": None}
    mod.set_axon_ntff_profile_hook = lambda h: state.__setitem__("hook", h)
    mod.get_axon_ntff_profile_hook = lambda: state["hook"]
    sys.modules["antenv.axon_hooks"] = mod
    antenv.axon_hooks = mod
    try:
        from trn_agent_boot.trn_boot import _ntff_profile_via_ctypes

        mod.set_axon_ntff_profile_hook(
            _ntff_profile_via_ctypes("/opt/axon/libaxon_pjrt.so")
        )
    except Exception:
        pass


def run(hidden_state, encoder_output, W1, b1, w2, trace=False):
    from concourse.bass_utils import run_bass_kernel_spmd

    if trace:
        _ensure_ntff_hook()

    nc = build_nc()
    nc.finalize()
    split_multi_waits(nc)
    in_maps = _prep_host(
        np.asarray(hidden_state, dtype=np.float32),
        np.asarray(encoder_output, dtype=np.float32),
        np.asarray(W1, dtype=np.float32),
        np.asarray(b1, dtype=np.float32),
        np.asarray(w2, dtype=np.float32),
    )
    res = run_bass_kernel_spmd(nc, in_maps, core_ids=list(range(N_CORES)), trace=trace)
    out = np.concatenate([res.results[i]["out"] for i in range(N_CORES)], axis=0)
    return out, res


def kernel(**inputs):
    out, _ = run(**inputs)
    return out


# revision 34
# speedup vs baseline: 1.3107x; 1.3107x over previous
"""Bahdanau-attention pooling kernel for TRN2, data-parallel over 8 NeuronCores.

Reference computation (per batch b):
    h   = tanh(enc @ W1enc.T + hid @ W1hid.T + b1)    [S, K]   (K = D = 512)
    e   = h @ w2                                       [S]
    a   = softmax(e)                                   [S]
    ctx = a @ enc                                      [D]

Distribution: batch dim (32) sharded 4-per-core across 8 cores; the small MLP
weights are replicated. No collectives needed.

Device algorithm (per core, single pass over the encoder stream):
  - encoder arrives pre-transposed [B_loc, D, S] so score matmuls contract
    over d on the partition dim with no on-chip transpose.
  - online (max-free) softmax: |e| <= ||w2||_1 ~= 11.3, so exp() never
    overflows fp32; accumulate unnormalized sum_s exp(e_s) * enc_s and
    z = sum_s exp(e_s), divide at the end.
  - per s-tile: 16 score MMs (k-chunk x d-chunk) -> tanh(+r bias) on ACT ->
    4 w2-MMs -> e row -> exp on ACT (accum_out gives the tile's z) ->
    replicate p across partitions with a K=1 ones-MM -> 4 fused
    multiply+reduce DVE ops accumulate the context.
"""

import numpy as np

B, S, D = 32, 4096, 512
N_CORES = 8
B_LOC = B // N_CORES
T = 512          # s-tile size
KC = D // 128    # 4 k-chunks
DC = D // 128    # 4 d-chunks


def build_nc(b_loc=B_LOC, s_len=S, t=T, dtype_name="float32"):
    import concourse.bass as bass
    import concourse.mybir as mybir
    import concourse.tile as tile

    fp32 = mybir.dt.float32
    bf16 = mybir.dt.bfloat16
    AF = mybir.ActivationFunctionType
    Alu = mybir.AluOpType

    nc = bass.Bass()

    enc_ext = nc.declare_dram_parameter("enc", [b_loc, D, s_len], bf16, isOutput=False)
    hid_ext = nc.declare_dram_parameter("hid", [b_loc, D], bf16, isOutput=False)
    w1et_ext = nc.declare_dram_parameter("w1et", [D, D], bf16, isOutput=False)
    w1ht_ext = nc.declare_dram_parameter("w1ht", [D, D], bf16, isOutput=False)
    b1_ext = nc.declare_dram_parameter("b1", [D], fp32, isOutput=False)
    w2_ext = nc.declare_dram_parameter("w2", [D], bf16, isOutput=False)
    out_ext = nc.declare_dram_parameter("out", [b_loc, D], fp32, isOutput=True)

    n_tiles = s_len // t

    with tile.TileContext(nc) as tc:
        with (
            tc.tile_pool(name="singles", bufs=1) as singles,
            tc.tile_pool(name="enc_pool", bufs=6) as enc_pool,
            tc.tile_pool(name="h_pool", bufs=12) as h_pool,
            tc.tile_pool(name="p_pool", bufs=4) as p_pool,
            tc.tile_pool(name="scr_pool", bufs=4) as scr_pool,
            tc.tile_pool(name="tiny", bufs=4) as tiny,
            tc.tile_pool(name="ps_h", bufs=3, space=bass.MemorySpace.PSUM) as ps_h,
            tc.tile_pool(name="ps_e", bufs=2, space=bass.MemorySpace.PSUM) as ps_e,
            tc.tile_pool(name="ps_p", bufs=2, space=bass.MemorySpace.PSUM) as ps_p,
            tc.tile_pool(name="ps_s", bufs=1, space=bass.MemorySpace.PSUM) as ps_s,
        ):
            # ---- persistent tiles (small loads first so r can start early) --
            hid_cols = singles.tile([128, b_loc, DC], bf16)  # [p(d), b, d-chunk]
            nc.gpsimd.dma_start(
                out=hid_cols, in_=hid_ext.rearrange("b (c p) -> p b c", p=128)
            )
            b1_col = singles.tile([128, KC], fp32)
            nc.gpsimd.dma_start(out=b1_col, in_=b1_ext.rearrange("(c p) -> p c", p=128))
            w2_col = singles.tile([128, KC], bf16)
            nc.gpsimd.dma_start(out=w2_col, in_=w2_ext.rearrange("(c p) -> p c", p=128))
            w1ht_sb = singles.tile([128, DC, D], bf16)
            nc.sync.dma_start(
                out=w1ht_sb, in_=w1ht_ext.rearrange("(c p) k -> p c k", p=128)
            )
            w1et_sb = singles.tile([128, DC, D], bf16)   # [p(d), d-chunk, k]
            nc.sync.dma_start(
                out=w1et_sb, in_=w1et_ext.rearrange("(c p) k -> p c k", p=128)
            )
            ones_row = singles.tile([1, 128], bf16)
            nc.vector.memset(ones_row, 1.0)
            ones_f32 = singles.tile([1, 128], fp32)
            nc.vector.memset(ones_f32, 1.0)

            r_sb = singles.tile([128, KC, b_loc], fp32)   # [p(k), k-chunk, b]
            c_acc = singles.tile([128, DC, b_loc], fp32)  # [p(d), d-chunk, b]
            z_acc = singles.tile([1, b_loc], fp32)
            nc.vector.memset(c_acc, 0.0)
            nc.vector.memset(z_acc, 0.0)

            # ---- r = W1hid @ hid + b1  (per k-chunk, all batches at once) ---
            for kc in range(KC):
                r_ps = ps_s.tile([128, b_loc], fp32, tag="s")
                for dc in range(DC):
                    nc.tensor.matmul(
                        r_ps,
                        w1ht_sb[:, dc, kc * 128:(kc + 1) * 128],
                        hid_cols[:, :, dc],
                        start=(dc == 0),
                        stop=(dc == DC - 1),
                    )
                nc.vector.tensor_scalar_add(
                    out=r_sb[:, kc, :], in0=r_ps, scalar1=b1_col[:, kc:kc + 1]
                )

            # ---- main loop: tiles processed in pairs so each LDWEIGHTS of a
            # (d-chunk, k-chunk) weight block feeds two matmuls -------------
            for b in range(b_loc):
                for it0 in range(0, n_tiles, 2):
                    encs = []
                    for j in range(2):
                        s0 = (it0 + j) * t
                        enc_t = enc_pool.tile([128, DC, t], bf16, tag="enc")
                        nc.sync.dma_start(
                            out=enc_t,
                            in_=enc_ext[b].rearrange("(c p) s -> p c s", p=128)[
                                :, :, s0:s0 + t
                            ],
                        )
                        encs.append(enc_t)

                    h_tiles = [[], []]
                    for kc in range(KC):
                        h_pss = [
                            ps_h.tile([128, t], fp32, tag="h", name=f"hps{j}")
                            for j in range(2)
                        ]
                        for dc in range(DC):
                            for j in range(2):
                                nc.tensor.matmul(
                                    h_pss[j],
                                    w1et_sb[:, dc, kc * 128:(kc + 1) * 128],
                                    encs[j][:, dc, :],
                                    start=(dc == 0),
                                    stop=(dc == DC - 1),
                                )
                        for j in range(2):
                            h_sb = h_pool.tile([128, t], bf16, tag="hsb")
                            nc.scalar.activation(
                                out=h_sb, in_=h_pss[j], func=AF.Tanh,
                                bias=r_sb[:, kc, b:b + 1], scale=1.0,
                            )
                            h_tiles[j].append(h_sb)

                    for j in range(2):
                        e_ps = ps_e.tile([1, t], fp32, tag="e")
                        for kc in range(KC):
                            nc.tensor.matmul(
                                e_ps,
                                w2_col[:, kc:kc + 1],
                                h_tiles[j][kc],
                                start=(kc == 0),
                                stop=(kc == KC - 1),
                            )

                        p_row = p_pool.tile([1, t], bf16, tag="p")
                        z_tile = tiny.tile([1, 1], fp32, tag="z")
                        nc.scalar.activation(
                            out=p_row, in_=e_ps, func=AF.Exp, accum_out=z_tile
                        )
                        nc.vector.tensor_tensor(
                            out=z_acc[:, b:b + 1], in0=z_acc[:, b:b + 1], in1=z_tile,
                            op=Alu.add,
                        )

                        prep_ps = ps_p.tile([128, t], fp32, tag="pr")
                        nc.tensor.matmul(
                            prep_ps, ones_row, p_row, start=True, stop=True
                        )

                        for dc in range(DC):
                            scr = scr_pool.tile([128, t], bf16, tag="scr")
                            ctmp = tiny.tile([128, 1], fp32, tag="ct")
                            nc.vector.scalar_tensor_tensor(
                                out=scr,
                                in0=encs[j][:, dc, :],
                                scalar=1.0,
                                in1=prep_ps,
                                op0=Alu.mult,
                                op1=Alu.mult,
                                accum_out=ctmp,
                            )
                            nc.vector.tensor_tensor(
                                out=c_acc[:, dc, b:b + 1],
                                in0=c_acc[:, dc, b:b + 1],
                                in1=ctmp,
                                op=Alu.add,
                            )

                # ---- batch epilogue -----------------------------------------
                zr = tiny.tile([1, 1], fp32)
                nc.vector.reciprocal(out=zr, in_=z_acc[:, b:b + 1])
                zr_ps = ps_s.tile([128, 1], fp32, tag="s")
                nc.tensor.matmul(zr_ps, ones_f32, zr, start=True, stop=True)
                out_t = tiny.tile([128, DC], fp32)
                nc.vector.tensor_scalar_mul(out=out_t, in0=c_acc[:, :, b], scalar1=zr_ps)
                nc.gpsimd.dma_start(
                    out=out_ext[b].rearrange("(c p) -> p c", p=128), in_=out_t
                )

    return nc


# Instruction opcodes whose ISA structs tolerate multi-waits (or that the
# split must not touch). Everything else on this walrus build has a single
# sync-wait slot, so excess waits move onto preceding same-engine NoOps.
_NO_SPLIT = {"EventSemaphore", "Call", "UnconditionalBranch", "RegisterMove"}


def split_multi_waits(nc, limit=1):
    import concourse.mybir as mybir

    ctr = 0
    for fn in nc.m.functions:
        for blk in fn.blocks:
            new = []
            for inst in blk.instructions:
                si = inst.sync_info
                waits = list(si.on_wait) if si is not None and si.on_wait else []
                if inst.opcode not in _NO_SPLIT and len(waits) > limit:
                    extra, keep = waits[:-limit], waits[-limit:]
                    for w in extra:
                        ctr += 1
                        new.append(mybir.InstNoOp(
                            name=f"WSPLIT-{ctr}", engine=inst.engine,
                            sync_info=mybir.SyncInfo(on_wait=[w], on_update=[])))
                    inst.sync_info = mybir.SyncInfo(
                        on_wait=keep,
                        on_update=list(si.on_update) if si.on_update else [])
                new.append(inst)
            blk.instructions = new
    return ctr


def _prep_host(hidden_state, encoder_output, W1, b1, w2):
    import ml_dtypes

    bf16 = ml_dtypes.bfloat16
    encT = np.ascontiguousarray(
        encoder_output.transpose(0, 2, 1).astype(bf16)
    )  # [B, D, S]
    w1et = np.ascontiguousarray(W1[:, :D].T.astype(bf16))   # [d, k]
    w1ht = np.ascontiguousarray(W1[:, D:].T.astype(bf16))   # [d, k]
    in_maps = []
    for i in range(N_CORES):
        sl = slice(i * B_LOC, (i + 1) * B_LOC)
        in_maps.append({
            "enc": np.ascontiguousarray(encT[sl]),
            "hid": np.ascontiguousarray(hidden_state[sl].astype(bf16)),
            "w1et": w1et,
            "w1ht": w1ht,
            "b1": np.ascontiguousarray(b1.astype(np.float32)),
            "w2": np.ascontiguousarray(w2.astype(bf16)),
        })
    return in_maps


def _ensure_ntff_hook():
    """Install the axon NTFF profile hook if the image lacks antenv.axon_hooks."""
    import sys
    import types

    try:
        import antenv.axon_hooks  # noqa: F401
        return
    except ImportError:
        pass
    import antenv

    mod = types.ModuleType("antenv.axon_hooks")
    state = {"hook": None}
    mod.set_axon_ntff_profile_hook = lambda h: state.__setitem__("hook", h)
    mod.get_axon_ntff_profile_hook = lambda: state["hook"]
    sys.modules["antenv.axon_hooks"] = mod
    antenv.axon_hooks = mod
    try:
        from trn_agent_boot.trn_boot import _ntff_profile_via_ctypes

        mod.set_axon_ntff_profile_hook(
            _ntff_profile_via_ctypes("/opt/axon/libaxon_pjrt.so")
        )
    except Exception:
        pass


def run(hidden_state, encoder_output, W1, b1, w2, trace=False):
    from concourse.bass_utils import run_bass_kernel_spmd

    if trace:
        _ensure_ntff_hook()

    nc = build_nc()
    nc.finalize()
    split_multi_waits(nc)
    in_maps = _prep_host(
        np.asarray(hidden_state, dtype=np.float32),
        np.asarray(encoder_output, dtype=np.float32),
        np.asarray(W1, dtype=np.float32),
        np.asarray(b1, dtype=np.float32),
        np.asarray(w2, dtype=np.float32),
    )
    res = run_bass_kernel_spmd(nc, in_maps, core_ids=list(range(N_CORES)), trace=trace)
    out = np.concatenate([res.results[i]["out"] for i in range(N_CORES)], axis=0)
    return out, res


def kernel(**inputs):
    out, _ = run(**inputs)
    return out
